# revision 2
# baseline (speedup 1.0000x reference)
"""ArcTanDistortion kernel for Trainium2 (8 NeuronCores, SPMD).

y = (2/pi) * atan(GAIN * x) / log(GAIN), elementwise over x of shape
(8, 2, 4194304) float32. Batch dim (8) is sharded across the 8 cores.

Traffic-minimized pipeline (16 MiB HBM per core vs 40 MiB for the f32
baseline): the host encodes x as fp8 e3m4 with a power-of-2 prescale
(16) folded into the ACT input scale, so the device reads 1 byte/elem.
Per tile: HWDGE DMA fp8 in -> ACT Arctan (scale GAIN/16) fp8->fp16 ->
DVE tensor_scalar multiply by QSCALE = 126/(pi/2) casting fp16->int8
(2x_2p DVE mode: all operands SBUF) -> int8 DMA out. Host decodes
int8 -> f32 with one constant multiply. Measured rel err 2.74e-3 vs
the 2e-2 gate.

ACT is the bottleneck engine (1 elem/cycle/lane @ 1.2 GHz = 54.6 us
per core for 8.39M elems), so the tile schedule is non-uniform: small
tiles at the start (ACT starts after a 0.25 MiB DMA) and end (short
DVE+out tail after the last ACT), big 8192-wide tiles in the middle
(fewer per-instruction overheads: 352 ACT cycles each).

Emission order: the in-DMA of tile n+3 is issued BEFORE the out-DMA of
tile n. Both share the SP HWDGE ring and out(n) carries a semaphore
wait on DVE(n); emitting it first would block input prefetch in SP
program order and cap lookahead.
"""

import numpy as np
import ml_dtypes

GAIN = 67.0
PRESCALE = 16.0                  # power of 2: exact on host, folded into ACT scale
ACT_SCALE = GAIN / PRESCALE
FP8_MAX = 15.5                   # e3m4 max normal (TRN FP8_EXP3, bias 3)
OUT_SCALE = float((2.0 / np.pi) / np.log(GAIN))
QSCALE = float(126.0 / (np.pi / 2.0))
DECODE = float(OUT_SCALE / QSCALE)

B, C, N = 8, 2, 4194304          # full input shape
PER_CORE = C * N                 # 8388608 elements per core
P = 128                          # SBUF partitions
L = PER_CORE // P                # 65536 free-dim elements per lane
# Non-uniform free-dim schedule, sums to L: short ramp-in/ramp-out.
MS = [2048, 4096] + [8192] * 7 + [1024, 1024]
assert sum(MS) == L
MMAX = max(MS)

N_CORES = 8
LOOKAHEAD = 3


def _build_nc(reps: int = 1):
    import concourse.bacc as bacc
    import concourse.mybir as mybir
    import concourse.tile as tile

    nc = bacc.Bacc()
    x_in = nc.dram_tensor("x", [PER_CORE], mybir.dt.float8e3, kind="ExternalInput")
    y_out = nc.dram_tensor("y", [PER_CORE], mybir.dt.int8, kind="ExternalOutput")

    offs = np.concatenate([[0], np.cumsum(MS)])  # per-lane offsets
    sched = [(int(offs[j]) * P, MS[j]) for j in range(len(MS))]
    NT = reps * len(sched)

    def tile_ap(t, j):
        start, m = sched[j % len(sched)]
        return t[start : start + P * m].rearrange("(p m) -> p m", p=P)

    with tile.TileContext(nc) as tc:
        with tc.tile_pool(name="in8", bufs=LOOKAHEAD + 1) as pin, tc.tile_pool(
            name="mid16", bufs=3
        ) as pmid, tc.tile_pool(name="out8", bufs=3) as pout:
            pending = {}

            def fetch(j):
                m = sched[j % len(sched)][1]
                t = pin.tile([P, m], mybir.dt.float8e3)
                nc.sync.dma_start(out=t[:], in_=tile_ap(x_in, j))
                pending[j] = t

            for j in range(min(LOOKAHEAD, NT)):
                fetch(j)
            for n in range(NT):
                m = sched[n % len(sched)][1]
                t8 = pending.pop(n)
                t16 = pmid.tile([P, m], mybir.dt.float16)
                nc.scalar.activation(
                    t16[:], t8[:], mybir.ActivationFunctionType.Arctan, scale=ACT_SCALE
                )
                o8 = pout.tile([P, m], mybir.dt.int8)
                nc.vector.tensor_scalar_mul(o8[:], t16[:], QSCALE)
                if n + LOOKAHEAD < NT:
                    fetch(n + LOOKAHEAD)
                nc.sync.dma_start(out=tile_ap(y_out, n), in_=o8[:])
    nc.finalize()
    return nc


_ENC_LUT = None


def _encode_fp8(x: np.ndarray) -> np.ndarray:
    """f32 -> e3m4 bytes of clip(x*PRESCALE, +-FP8_MAX), via an f16-bit LUT."""
    global _ENC_LUT
    if _ENC_LUT is None:
        bits = np.arange(65536, dtype=np.uint16)
        vals = bits.view(np.float16).astype(np.float32)
        vals = np.clip(vals * np.float32(PRESCALE), -FP8_MAX, FP8_MAX)
        vals = np.nan_to_num(vals, nan=0.0, posinf=FP8_MAX, neginf=-FP8_MAX)
        _ENC_LUT = vals.astype(ml_dtypes.float8_e3m4).view(np.uint8)
    f16 = x.astype(np.float16).view(np.uint16)
    return _ENC_LUT[f16].view(ml_dtypes.float8_e3m4)


_NC_CACHE = None


def kernel(x: np.ndarray) -> np.ndarray:
    global _NC_CACHE
    from concourse.bass_utils import run_bass_kernel_spmd

    x = np.asarray(x, dtype=np.float32)
    assert x.shape == (B, C, N), x.shape

    enc = _encode_fp8(np.ascontiguousarray(x).reshape(B, PER_CORE))

    if _NC_CACHE is None:
        _NC_CACHE = _build_nc()
    nc = _NC_CACHE
    in_maps = [{"x": enc[i]} for i in range(N_CORES)]
    # The axon-proxied LoadExecutable occasionally fails transiently right
    # after another process released the cores; retry a couple of times.
    last_err = None
    for attempt in range(3):
        try:
            rr = run_bass_kernel_spmd(nc, in_maps, list(range(N_CORES)))
            break
        except Exception as e:  # noqa: BLE001 - retry any runtime load failure
            last_err = e
            import time as _time

            _time.sleep(5.0 * (attempt + 1))
    else:
        raise last_err

    out = np.empty((B, C, N), dtype=np.float32)
    for i in range(N_CORES):
        out[i] = rr.results[i]["y"].astype(np.float32).reshape(C, N) * np.float32(
            DECODE
        )
    return out


# revision 5
# speedup vs baseline: 6.8937x; 6.8937x over previous
"""ArcTanDistortion kernel for Trainium2 (8 NeuronCores, SPMD).

y = (2/pi) * atan(GAIN * x) / log(GAIN), elementwise over x of shape
(8, 2, 4194304) float32. Batch dim (8) is sharded across the 8 cores.

Traffic-minimized pipeline (16 MiB HBM per core vs 40 MiB for the f32
baseline): the host encodes x as fp8 e3m4 with a power-of-2 prescale
(16) folded into the ACT input scale, so the device reads 1 byte/elem.
Per tile: HWDGE DMA fp8 in -> ACT Arctan (scale GAIN/16) fp8->fp16 ->
DVE tensor_scalar multiply by QSCALE = 126/(pi/2) casting fp16->int8
(2x_2p DVE mode: all operands SBUF) -> int8 DMA out. Host decodes
int8 -> f32 with one constant multiply. Measured rel err 2.74e-3 vs
the 2e-2 gate.

ACT is the bottleneck engine (1 elem/cycle/lane @ 1.2 GHz = 54.6 us
per core for 8.39M elems). Uniform [128, 8192] tiles: measured best
steady-state slope on HW (~51 us/pass vs ~60 us for schedules with
small ramp/tail tiles, whose short in-DMAs give the prefetch window
too little time-cover against the ~2 us DMA completion latency).

Emission order: the in-DMA of tile n+3 is issued BEFORE the out-DMA of
tile n. Both share the SP HWDGE ring and out(n) carries a semaphore
wait on DVE(n); emitting it first would block input prefetch in SP
program order and cap lookahead.
"""

import numpy as np
import ml_dtypes

GAIN = 67.0
PRESCALE = 16.0                  # power of 2: exact on host, folded into ACT scale
ACT_SCALE = GAIN / PRESCALE
FP8_MAX = 15.5                   # e3m4 max normal (TRN FP8_EXP3, bias 3)
OUT_SCALE = float((2.0 / np.pi) / np.log(GAIN))
QSCALE = float(126.0 / (np.pi / 2.0))
DECODE = float(OUT_SCALE / QSCALE)

B, C, N = 8, 2, 4194304          # full input shape
PER_CORE = C * N                 # 8388608 elements per core
P = 128                          # SBUF partitions
L = PER_CORE // P                # 65536 free-dim elements per lane
MS = [8192] * 8                  # free-dim tile schedule, sums to L
assert sum(MS) == L

N_CORES = 8
LOOKAHEAD = 3


def _build_nc(reps: int = 1):
    import concourse.bacc as bacc
    import concourse.mybir as mybir
    import concourse.tile as tile

    nc = bacc.Bacc()
    x_in = nc.dram_tensor("x", [PER_CORE], mybir.dt.float8e3, kind="ExternalInput")
    y_out = nc.dram_tensor("y", [PER_CORE], mybir.dt.int8, kind="ExternalOutput")

    offs = np.concatenate([[0], np.cumsum(MS)])  # per-lane offsets
    sched = [(int(offs[j]) * P, MS[j]) for j in range(len(MS))]
    NT = reps * len(sched)

    def tile_ap(t, j):
        start, m = sched[j % len(sched)]
        return t[start : start + P * m].rearrange("(p m) -> p m", p=P)

    with tile.TileContext(nc) as tc:
        with tc.tile_pool(name="in8", bufs=LOOKAHEAD + 1) as pin, tc.tile_pool(
            name="mid16", bufs=3
        ) as pmid, tc.tile_pool(name="out8", bufs=3) as pout:
            pending = {}

            def fetch(j):
                m = sched[j % len(sched)][1]
                t = pin.tile([P, m], mybir.dt.float8e3)
                nc.sync.dma_start(out=t[:], in_=tile_ap(x_in, j))
                pending[j] = t

            for j in range(min(LOOKAHEAD, NT)):
                fetch(j)
            for n in range(NT):
                m = sched[n % len(sched)][1]
                t8 = pending.pop(n)
                t16 = pmid.tile([P, m], mybir.dt.float16)
                nc.scalar.activation(
                    t16[:], t8[:], mybir.ActivationFunctionType.Arctan, scale=ACT_SCALE
                )
                o8 = pout.tile([P, m], mybir.dt.int8)
                nc.vector.tensor_scalar_mul(o8[:], t16[:], QSCALE)
                if n + LOOKAHEAD < NT:
                    fetch(n + LOOKAHEAD)
                nc.sync.dma_start(out=tile_ap(y_out, n), in_=o8[:])
    nc.finalize()
    return nc


_ENC_LUT = None


def _encode_fp8(x: np.ndarray) -> np.ndarray:
    """f32 -> e3m4 bytes of clip(x*PRESCALE, +-FP8_MAX), via an f16-bit LUT."""
    global _ENC_LUT
    if _ENC_LUT is None:
        bits = np.arange(65536, dtype=np.uint16)
        vals = bits.view(np.float16).astype(np.float32)
        with np.errstate(invalid="ignore"):
            vals = np.clip(vals * np.float32(PRESCALE), -FP8_MAX, FP8_MAX)
        vals = np.nan_to_num(vals, nan=0.0, posinf=FP8_MAX, neginf=-FP8_MAX)
        _ENC_LUT = vals.astype(ml_dtypes.float8_e3m4).view(np.uint8)
    f16 = x.astype(np.float16).view(np.uint16)
    return _ENC_LUT[f16].view(ml_dtypes.float8_e3m4)


_NC_CACHE = None


def kernel(x: np.ndarray) -> np.ndarray:
    global _NC_CACHE
    from concourse.bass_utils import run_bass_kernel_spmd

    x = np.asarray(x, dtype=np.float32)
    assert x.shape == (B, C, N), x.shape

    enc = _encode_fp8(np.ascontiguousarray(x).reshape(B, PER_CORE))

    if _NC_CACHE is None:
        _NC_CACHE = _build_nc()
    nc = _NC_CACHE
    in_maps = [{"x": enc[i]} for i in range(N_CORES)]
    # The axon-proxied LoadExecutable occasionally fails transiently right
    # after another process released the cores; retry a couple of times.
    last_err = None
    for attempt in range(3):
        try:
            rr = run_bass_kernel_spmd(nc, in_maps, list(range(N_CORES)))
            break
        except Exception as e:  # noqa: BLE001 - retry any runtime load failure
            last_err = e
            import time as _time

            _time.sleep(5.0 * (attempt + 1))
    else:
        raise last_err

    out = np.empty((B, C, N), dtype=np.float32)
    for i in range(N_CORES):
        out[i] = rr.results[i]["y"].astype(np.float32).reshape(C, N) * np.float32(
            DECODE
        )
    return out
